# revision 60
# baseline (speedup 1.0000x reference)
"""DeltaNet-style fast-weight kernel for Trainium2 (8 NeuronCores, data-parallel over batch).

Math (per batch element b):
  h_t = LN(e + MLP(e))[seq_t]  -- pure per-token function of seq_t (64 distinct values!)
  keys k_t = h_t, t=0..510 ; kn_t = k_t/||k_t||
  M_t = M_{t-1}(I - kn_t kn_t^T) + k_t kn_t^T ; y = M_510 @ h_511
  out = (y @ rp_w + rp_b) @ out_w + out_b

Key structural reductions vs a naive implementation:
  1. Since h_t depends only on the token id, the entire embed+MLP+LN
     pipeline collapses to 64-row token tables T_h (keys) and T_kn
     (normalized keys) computed once on-chip (64 rows, not B*L tokens).
  2. Per-token key rows are gathered with tiny one-hot PE matmuls
     (lhsT = onehot [64v,32b], rhs = T_kn) into per-16-step chunks,
     produced in REVERSE time order so they pipeline under the scan.
  3. y = sum_t c_t k_t with c_t from the backward vector scan
       z_{510} = q;  c_t = kn_t . z_t;  z_{t-1} = z_t - c_t kn_t
     (z tracked negated; 2 fused DVE ops per step = the critical path;
     ~58+64 DVE cycles per op is the ISA floor for this recurrence).
  4. The y accumulation runs on the otherwise-idle Pool engine in a
     4-phase layout (partition (t%4)*32+b): unnormalized keys T_h[seq_t]
     land there via one extra PE matmul per 4 steps, c is rephased by
     small SBUF-SBUF DMAs, Pool does tensor_scalar+tensor_tensor per
     4 steps, and one PE matmul folds the 4 phases at the end.
"""

import os
import sys

import numpy as np

for _p in ("/opt/trn_rl_repo", "/root/.axon_site/_ro/trn_rl_repo"):
    if os.path.isdir(_p) and _p not in sys.path:
        sys.path.insert(0, _p)

import concourse.bass as bass
import concourse.tile as tile
from concourse import bacc, mybir
from concourse.bass_utils import run_bass_kernel_spmd
from concourse.masks import make_identity

F32 = mybir.dt.float32
I32 = mybir.dt.int32
AF = mybir.ActivationFunctionType
OP = mybir.AluOpType

B, L, H, V = 256, 512, 64, 64
NCORES = 8
BS = B // NCORES          # 32 batches per core
CH = 16                   # t-steps per pipeline chunk
NCH = L // CH             # 32 chunks
LN_EPS = 1e-5


def _ap_bcast(dram_ap, nparts):
    """Partition-broadcast a DRAM AP across nparts partitions."""
    return bass.AP(
        tensor=dram_ap.tensor,
        offset=dram_ap.offset,
        ap=[[0, nparts], *dram_ap.ap],
    )


def build_program():
    nc = bacc.Bacc(None, target_bir_lowering=False)

    seq_p = nc.declare_dram_parameter("seq", [BS, L], I32, isOutput=False)
    embed_p = nc.declare_dram_parameter("embed", [V, H], F32, isOutput=False)
    w1_p = nc.declare_dram_parameter("w1", [H, 2 * H], F32, isOutput=False)
    b1_p = nc.declare_dram_parameter("b1", [2 * H, 1], F32, isOutput=False)
    w2_p = nc.declare_dram_parameter("w2", [2 * H, H], F32, isOutput=False)
    b2_p = nc.declare_dram_parameter("b2", [1, H], F32, isOutput=False)
    ln_g_p = nc.declare_dram_parameter("ln_g", [1, H], F32, isOutput=False)
    ln_b_p = nc.declare_dram_parameter("ln_b", [1, H], F32, isOutput=False)
    rp_w_p = nc.declare_dram_parameter("rp_w", [H, H], F32, isOutput=False)
    rp_b_p = nc.declare_dram_parameter("rp_b", [H, 1], F32, isOutput=False)
    out_w_p = nc.declare_dram_parameter("out_w", [H, V], F32, isOutput=False)
    out_b_p = nc.declare_dram_parameter("out_b", [1, V], F32, isOutput=False)
    out_p = nc.declare_dram_parameter("out", [BS, V], F32, isOutput=True)

    # DRAM bounce for seq^T so per-chunk partition-broadcast DMAs read
    # contiguous runs.
    seqT_d = nc.dram_tensor("seqT_scratch", [L, BS], F32)

    from contextlib import ExitStack

    with tile.TileContext(nc) as tc, ExitStack() as ctx:
        consts = ctx.enter_context(tc.tile_pool(name="consts", bufs=1))
        state = ctx.enter_context(tc.tile_pool(name="state", bufs=1))
        ohp = ctx.enter_context(tc.tile_pool(name="ohp", bufs=4))
        sqp = ctx.enter_context(tc.tile_pool(name="sqp", bufs=4))
        knp = ctx.enter_context(tc.tile_pool(name="knp", bufs=6))
        ps_g = ctx.enter_context(tc.tile_pool(name="ps_g", bufs=2, space="PSUM"))
        ps_ph = ctx.enter_context(tc.tile_pool(name="ps_ph", bufs=1, space="PSUM"))
        ps_m = ctx.enter_context(tc.tile_pool(name="ps_m", bufs=1, space="PSUM"))

        # ---------------- constants / params ----------------
        ident = consts.tile([H, H], F32)
        make_identity(nc, ident)

        eps_sb = consts.tile([V, 1], F32)
        nc.vector.memset(eps_sb, LN_EPS)
        ones1 = consts.tile([1, BS], F32)
        nc.vector.memset(ones1, 1.0)

        viota_i = consts.tile([V, 1], I32)
        nc.gpsimd.iota(viota_i, pattern=[[1, 1]], base=0, channel_multiplier=1)
        viota = consts.tile([V, 1], F32)
        nc.vector.tensor_copy(viota, viota_i)

        # seq load + transpose chain on the SP DMA queue (ahead of params so
        # the per-chunk broadcasts start early); param loads ride the Act
        # engine's DMA queue in parallel.
        seq_hi = consts.tile([BS, 128], I32)
        seq_lo = consts.tile([BS, 384], I32)
        nc.sync.dma_start(out=seq_hi, in_=seq_p[:, 384:512])

        embed_sb = consts.tile([V, H], F32)
        w1_sb = consts.tile([H, 2 * H], F32)
        b1_sb = consts.tile([2 * H, 1], F32)
        w2_sb = consts.tile([2 * H, H], F32)
        rp_w_sb = consts.tile([H, H], F32)
        rp_b_sb = consts.tile([H, 1], F32)
        out_w_sb = consts.tile([H, V], F32)
        out_b_sb = consts.tile([1, V], F32)
        b2_bc = consts.tile([V, H], F32)
        g_bc = consts.tile([V, H], F32)
        bta_bc = consts.tile([V, H], F32)
        # table weights split across the two fast DMA queues so none lands
        # later than ~3us: embed/w1 behind nothing on Act, b1/w2 behind one
        # small seq quarter on SP.
        nc.scalar.dma_start(out=embed_sb, in_=embed_p[:, :])
        nc.scalar.dma_start(out=w1_sb, in_=w1_p[:, :])
        nc.sync.dma_start(out=b1_sb, in_=b1_p[:, :])
        nc.sync.dma_start(out=w2_sb, in_=w2_p[:, :])
        nc.sync.dma_start(out=seq_lo, in_=seq_p[:, 0:384])
        # mid-table broadcast params go through the gpsimd SWDGE queue,
        # which is idle until the first one-hot op.
        nc.gpsimd.dma_start(out=b2_bc, in_=_ap_bcast(b2_p[0, :], V))
        nc.gpsimd.dma_start(out=g_bc, in_=_ap_bcast(ln_g_p[0, :], V))
        nc.gpsimd.dma_start(out=bta_bc, in_=_ap_bcast(ln_b_p[0, :], V))

        # Touch every activation function used later so the ACT table loads
        # (~1.3us each) happen now, overlapped with the DMA transfers --
        # emitted AFTER the dma_start issues so they don't delay them.
        act_warm = consts.tile([V, 1], F32)
        nc.scalar.activation(act_warm, eps_sb, AF.Sqrt)
        nc.scalar.activation(act_warm, eps_sb, AF.Relu)
        nc.scalar.activation(act_warm, eps_sb, AF.Identity, bias=eps_sb[:, 0:1])
        nc.scalar.activation(act_warm, eps_sb, AF.Copy)

        # PE warm-up: a chain of throwaway matmuls keeps the PE p-state ramp
        # going while the parameter DMAs land, so the first gather matmuls run
        # at full clock. Depends only on the gpsimd-built identity.
        dummy_ps = ps_m.tile([BS, BS], F32, tag="psm")
        nc.tensor.matmul(dummy_ps, lhsT=ident[0:BS, 0:BS], rhs=ident[0:BS, 0:BS], start=True, stop=True)
        warm_ps = ps_m.tile([H, H], F32, tag="warm")
        for _ in range(10):
            nc.tensor.matmul(warm_ps, lhsT=ident, rhs=ident, start=True, stop=True)

        # seq -> f32, transpose via PE, bounce to DRAM (all ahead of tables).
        # Processed k descending so the last-time quarter (which the reverse
        # pipeline consumes first) reaches DRAM earliest; the k=3 quarter has
        # its own tiles so it never waits on the k<3 loads.
        seq_fhi = consts.tile([BS, 128], F32)
        seq_flo = consts.tile([BS, 384], F32)
        seqT_hi = consts.tile([128, BS], F32)
        seqT_lo = consts.tile([128, 3, BS], F32)
        nc.vector.tensor_copy(seq_fhi, seq_hi)
        pst = ps_m.tile([128, BS], F32, tag="psm")
        nc.tensor.matmul(pst, lhsT=seq_fhi, rhs=ident[0:BS, 0:BS], start=True, stop=True)
        nc.vector.tensor_copy(seqT_hi, pst)
        nc.sync.dma_start(out=seqT_d[384:512, :], in_=seqT_hi)
        nc.vector.tensor_copy(seq_flo, seq_lo)
        for k in range(2, -1, -1):
            pst = ps_m.tile([128, BS], F32, tag="psm")
            nc.tensor.matmul(pst, lhsT=seq_flo[:, 128 * k:128 * (k + 1)], rhs=ident[0:BS, 0:BS], start=True, stop=True)
            nc.vector.tensor_copy(seqT_lo[:, k, :], pst)
            nc.sync.dma_start(
                out=seqT_d[128 * k:128 * (k + 1), :],
                in_=seqT_lo[:, k, :],
            )

        # ---------------- token tables ----------------
        # eT = embed^T
        psE = ps_m.tile([H, V], F32, tag="psm")
        nc.tensor.matmul(psE, lhsT=embed_sb, rhs=ident, start=True, stop=True)
        eT_sb = consts.tile([H, V], F32)
        nc.scalar.activation(eT_sb, psE, AF.Copy)

        # a1T = (e @ w1)^T  [2H, V], relu(+b1)
        psA = ps_m.tile([2 * H, V], F32, tag="psm")
        nc.tensor.matmul(psA, lhsT=w1_sb, rhs=eT_sb, start=True, stop=True)
        rT = consts.tile([2 * H, V], F32)
        nc.scalar.activation(rT, psA, AF.Relu, bias=b1_sb[:, 0:1])

        # x = e + a1 @ w2 + b2   [V tokens, H]
        psX = ps_m.tile([V, H], F32, tag="psm")
        nc.tensor.matmul(psX, lhsT=rT, rhs=w2_sb, start=True, stop=True)
        x_sb = consts.tile([V, H], F32)
        nc.scalar.activation(x_sb, psX, AF.Copy)
        nc.vector.tensor_add(x_sb, x_sb, embed_sb)
        nc.vector.tensor_add(x_sb, x_sb, b2_bc)

        # LayerNorm over H (free axis)
        st6 = consts.tile([V, 6], F32)
        mv = consts.tile([V, 2], F32)
        nc.vector.bn_stats(st6, x_sb)
        nc.vector.bn_aggr(mv, st6)
        sstd = consts.tile([V, 1], F32)
        rstd = consts.tile([V, 1], F32)
        nc.scalar.activation(sstd, mv[:, 1:2], AF.Sqrt, bias=eps_sb[:, 0:1])
        nc.vector.reciprocal(rstd, sstd)
        T_h = consts.tile([V, H], F32)
        nc.vector.tensor_scalar(
            out=T_h, in0=x_sb, scalar1=mv[:, 0:1], scalar2=rstd[:, 0:1],
            op0=OP.subtract, op1=OP.mult,
        )
        nc.vector.tensor_mul(T_h, T_h, g_bc)
        nc.vector.tensor_add(T_h, T_h, bta_bc)

        # Row norms; T_kn = T_h / max(||T_h||, 1e-12)
        ssq = consts.tile([V, 1], F32)
        scr = consts.tile([V, H], F32)
        nc.vector.scalar_tensor_tensor(
            out=scr, in0=T_h, scalar=1.0, in1=T_h,
            op0=OP.mult, op1=OP.mult, accum_out=ssq[:, 0:1],
        )
        snrm = consts.tile([V, 1], F32)
        nc.scalar.activation(snrm, ssq, AF.Sqrt)
        nc.vector.tensor_scalar(snrm, snrm, 1e-12, None, op0=OP.max)
        rnrm = consts.tile([V, 1], F32)
        nc.vector.reciprocal(rnrm, snrm)

        T_kn = consts.tile([V, H], F32)
        nc.vector.tensor_scalar(
            out=T_kn, in0=T_h, scalar1=rnrm[:, 0:1], scalar2=None,
            op0=OP.mult,
        )

        # stacked identity [128, 32]: row (ph, b) has a 1 in column b.
        # (Filled by DMA later, after the time-critical chunk DMAs are queued.)
        fold_id = consts.tile([128, BS], F32)

        # ---------------- state ----------------
        zneg = state.tile([BS, H], F32)
        u = state.tile([BS, H], F32)
        c_sb = state.tile([BS, L], F32)
        nc.vector.memset(c_sb, 0.0)
        # phased y accumulation: partition (t%4)*32+b, column t//4
        kh_ph = state.tile([128, L // 4, H], F32)   # unnormalized keys, phased
        c_ph = state.tile([128, L // 4], F32)
        y4 = state.tile([128, H], F32)
        u4 = state.tile([128, H], F32)
        y0 = state.tile([BS, H], F32)
        nc.gpsimd.memset(y4, 0.0)

        # ---------------- reverse-order pipeline: gather + scan + y ----------
        for ci in range(NCH - 1, -1, -1):
            t0 = CH * ci

            sqb = sqp.tile([V, CH * BS], F32)
            nc.sync.dma_start(out=sqb, in_=_ap_bcast(seqT_d[t0:t0 + CH, :], V))

            if ci == NCH - 2:
                for sb, p in (
                    (rp_w_sb, rp_w_p), (rp_b_sb, rp_b_p),
                    (out_w_sb, out_w_p), (out_b_sb, out_b_p),
                ):
                    nc.sync.dma_start(out=sb, in_=p[:, :])

            oh = ohp.tile([V, CH, BS], F32)
            nc.gpsimd.tensor_scalar(
                out=oh, in0=sqb.rearrange("v (t b) -> v t b", t=CH), scalar1=viota[:, 0:1], scalar2=None,
                op0=OP.is_equal,
            )

            if ci == NCH - 1:
                # q = h[:, 511, :] (unnormalized), zneg = -q
                psQ = ps_ph.tile([BS, H], F32, tag="psQ")
                nc.tensor.matmul(psQ, lhsT=oh[:, CH - 1, :], rhs=T_h, start=True, stop=True)
                nc.vector.tensor_scalar(
                    out=zneg, in0=psQ, scalar1=-1.0, scalar2=None, op0=OP.mult,
                )

            # gathers in reverse time order; evacuated piecewise (high steps
            # first) so the scan can begin before the rest lands. Quarter
            # granularity for the very first chunk (startup critical path).
            ev = 4 if ci == NCH - 1 else CH // 2
            psG = ps_g.tile([BS, CH, H], F32, tag="psG")
            knh = knp.tile([BS, CH, H], F32)
            for j in range(CH - 1, -1, -1):
                nc.tensor.matmul(
                    psG[:, j, :], lhsT=oh[:, j, :], rhs=T_kn,
                    start=True, stop=True,
                )
                if j % ev == 0:
                    nc.scalar.activation(
                        knh[:, j:j + ev, :], psG[:, j:j + ev, :], AF.Copy,
                    )

            if ci == 0:
                # unnormalized keys for the final chunk: its y contribution
                # is accumulated on DVE right after the scan (the Pool path's
                # c-rephase DMA would land too late).
                psG0 = ps_g.tile([BS, CH, H], F32, tag="psG")
                kh0 = knp.tile([BS, CH, H], F32)
                for j in range(CH - 1, -1, -1):
                    nc.tensor.matmul(
                        psG0[:, j, :], lhsT=oh[:, j, :], rhs=T_h,
                        start=True, stop=True,
                    )
                nc.scalar.activation(kh0, psG0, AF.Copy)

            # scan steps t = t0+CH-1 .. t0 (t=511 is the query, not a key)
            for j in range(CH - 1, -1, -1):
                t = t0 + j
                if t == L - 1:
                    continue
                kn_ap = knh[:, j, 0:H]
                nc.vector.scalar_tensor_tensor(
                    out=u, in0=kn_ap, scalar=-1.0, in1=zneg,
                    op0=OP.mult, op1=OP.mult, accum_out=c_sb[:, t:t + 1],
                )
                nc.vector.scalar_tensor_tensor(
                    out=zneg, in0=kn_ap, scalar=c_sb[:, t:t + 1], in1=zneg,
                    op0=OP.mult, op1=OP.add,
                )

            # unnormalized keys in phased layout (partition (t%4)*32+b), via
            # one PE matmul per 4 steps; feeds the Pool y-accumulation.
            psP = ps_ph.tile([128, 4, H], F32, tag="psP")
            for g in range(4):
                nc.tensor.matmul(
                    psP[:, g, :],
                    lhsT=oh[:, 4 * g:4 * g + 4, :].rearrange("v t b -> v (t b)"),
                    rhs=T_h, start=True, stop=True,
                )
            nc.scalar.activation(kh_ph[:, 4 * ci:4 * ci + 4, :], psP, AF.Copy)

            if ci == 20:
                # fill fold_id while the SP queue is past its critical phase
                for ph in range(4):
                    nc.sync.dma_start(
                        out=fold_id[32 * ph:32 * (ph + 1), :],
                        in_=ident[0:BS, 0:BS],
                    )

            # every 2 chunks (32 steps; singly for the last two, to shrink
            # the end-of-kernel tail): phase-rearrange c via DMA, then
            # y4 += c_ph[col] * kh_ph[col] on Pool (trailing the scan).
            # c is directly the coefficient of the unnormalized key k_t:
            # y = sum_t c_t k_t (t=511 slot stays zero-filled).
            if ci >= 2 and ci % 2 == 0:
                cols0, ncol, tspan = 4 * ci, 8, 2 * CH
            elif ci == 1:
                cols0, ncol, tspan = 4 * ci, 4, CH
            else:
                cols0 = None
            if cols0 is not None:
                for ph in range(4):
                    nc.sync.dma_start(
                        out=c_ph[32 * ph:32 * (ph + 1), cols0:cols0 + ncol],
                        in_=c_sb[:, t0 + ph:t0 + tspan:4],
                    )
                for col in range(cols0, cols0 + ncol):
                    nc.gpsimd.tensor_scalar(
                        out=u4, in0=kh_ph[:, col, :],
                        scalar1=c_ph[:, col:col + 1], scalar2=None, op0=OP.mult,
                    )
                    nc.gpsimd.tensor_tensor(out=y4, in0=u4, in1=y4, op=OP.add)

            if ci == 0:
                # final chunk's y on DVE (t = 0..15 are all keys)
                nc.vector.tensor_scalar(
                    out=y0, in0=kh0[:, CH - 1, :],
                    scalar1=c_sb[:, CH - 1:CH], scalar2=None, op0=OP.mult,
                )
                for j in range(CH - 2, -1, -1):
                    nc.vector.scalar_tensor_tensor(
                        out=y0, in0=kh0[:, j, :], scalar=c_sb[:, j:j + 1],
                        in1=y0, op0=OP.mult, op1=OP.add,
                    )

        # ---------------- final projections ----------------
        # Precompute the merged projection W_RO = rp_w @ out_w and its bias
        # row B_ROr = rp_b^T out_w + out_b (PE is idle by now), collapsing
        # both output matmuls and the bias adds into one PSUM accumulation.
        psT = ps_m.tile([H, H], F32, tag="psm")
        nc.tensor.matmul(psT, lhsT=rp_w_sb, rhs=ident, start=True, stop=True)
        rp_wT = state.tile([H, H], F32)
        nc.vector.tensor_copy(rp_wT, psT)
        psW = ps_m.tile([H, V], F32, tag="psm")
        nc.tensor.matmul(psW, lhsT=rp_wT, rhs=out_w_sb, start=True, stop=True)
        W_RO = state.tile([H, V], F32)
        nc.vector.tensor_copy(W_RO, psW)
        psB = ps_m.tile([1, V], F32, tag="psm")
        nc.tensor.matmul(psB, lhsT=rp_b_sb, rhs=out_w_sb, start=True, stop=True)
        B_ROr = state.tile([1, V], F32)
        nc.vector.tensor_add(B_ROr, psB, out_b_sb)

        # fold the 4 phase groups of y4 and transpose in one matmul, and
        # accumulate the last chunk's DVE-side y0 (transposed) on top:
        # yT[h, b] = sum_(ph,b') y4[(ph,b'), h] * FOLD[(ph,b'), b] + y0[b, h]
        psF = ps_m.tile([H, BS], F32, tag="psm")
        nc.tensor.matmul(psF, lhsT=y4, rhs=fold_id, start=True, stop=False)
        nc.tensor.matmul(psF, lhsT=y0, rhs=ident[0:BS, 0:BS], start=False, stop=True)
        yT = state.tile([H, BS], F32)
        nc.vector.tensor_copy(yT, psF)

        # out[b, v] = sum_h yT[h, b] W_RO[h, v] + B_ROr[v] -- the bias rides
        # a second contract-1 matmul against a ones column into the same PSUM
        psO = ps_m.tile([BS, V], F32, tag="psm")
        nc.tensor.matmul(psO, lhsT=yT, rhs=W_RO, start=True, stop=False)
        nc.tensor.matmul(psO, lhsT=ones1, rhs=B_ROr, start=False, stop=True)
        o_sb = state.tile([BS, V], F32)
        nc.vector.tensor_copy(o_sb, psO)
        nc.sync.dma_start(out=out_p[:, :], in_=o_sb)

    nc.finalize()
    return nc


_CACHE = {}


def _run(inputs, trace=False, **kw):
    seq = np.asarray(inputs["seq"]).astype(np.int32)
    embed = np.asarray(inputs["embed"], np.float32)
    w1 = np.asarray(inputs["w1"], np.float32)
    b1 = np.asarray(inputs["b1"], np.float32).reshape(2 * H, 1)
    w2 = np.asarray(inputs["w2"], np.float32)
    b2 = np.asarray(inputs["b2"], np.float32).reshape(1, H)
    ln_g = np.asarray(inputs["ln_g"], np.float32).reshape(1, H)
    ln_b = np.asarray(inputs["ln_b"], np.float32).reshape(1, H)
    rp_w = np.asarray(inputs["rp_w"], np.float32)
    rp_b = np.asarray(inputs["rp_b"], np.float32).reshape(H, 1)
    out_w = np.asarray(inputs["out_w"], np.float32)
    out_b = np.asarray(inputs["out_b"], np.float32).reshape(1, V)

    if "prog" not in _CACHE:
        _CACHE["prog"] = build_program()
    nc = _CACHE["prog"]

    in_maps = []
    for c in range(NCORES):
        in_maps.append({
            "seq": seq[BS * c:BS * (c + 1)],
            "embed": embed, "w1": w1, "b1": b1, "w2": w2, "b2": b2,
            "ln_g": ln_g, "ln_b": ln_b,
            "rp_w": rp_w, "rp_b": rp_b, "out_w": out_w, "out_b": out_b,
        })
    br = run_bass_kernel_spmd(nc, in_maps, list(range(NCORES)), trace=trace, **kw)
    out = np.concatenate([r["out"] for r in br.results], axis=0)
    return out, br


def kernel(**inputs) -> np.ndarray:
    return _run(inputs)[0]


# revision 61
# speedup vs baseline: 1.0020x; 1.0020x over previous
"""DeltaNet-style fast-weight kernel for Trainium2 (8 NeuronCores, data-parallel over batch).

Math (per batch element b):
  h_t = LN(e + MLP(e))[seq_t]  -- pure per-token function of seq_t (64 distinct values!)
  keys k_t = h_t, t=0..510 ; kn_t = k_t/||k_t||
  M_t = M_{t-1}(I - kn_t kn_t^T) + k_t kn_t^T ; y = M_510 @ h_511
  out = (y @ rp_w + rp_b) @ out_w + out_b

Key structural reductions vs a naive implementation:
  1. Since h_t depends only on the token id, the entire embed+MLP+LN
     pipeline collapses to 64-row token tables T_h (keys) and T_kn
     (normalized keys) computed once on-chip (64 rows, not B*L tokens).
  2. Per-token key rows are gathered with tiny one-hot PE matmuls
     (lhsT = onehot [64v,32b], rhs = T_kn) into per-16-step chunks,
     produced in REVERSE time order so they pipeline under the scan.
  3. y = sum_t c_t k_t with c_t from the backward vector scan
       z_{510} = q;  c_t = kn_t . z_t;  z_{t-1} = z_t - c_t kn_t
     (z tracked negated; 2 fused DVE ops per step = the critical path;
     ~58+64 DVE cycles per op is the ISA floor for this recurrence).
  4. The y accumulation runs on the otherwise-idle Pool engine in a
     4-phase layout (partition (t%4)*32+b): unnormalized keys T_h[seq_t]
     land there via one extra PE matmul per 4 steps, c is rephased by
     small SBUF-SBUF DMAs, Pool does tensor_scalar+tensor_tensor per
     4 steps, and one PE matmul folds the 4 phases at the end.
"""

import os
import sys

import numpy as np

for _p in ("/opt/trn_rl_repo", "/root/.axon_site/_ro/trn_rl_repo"):
    if os.path.isdir(_p) and _p not in sys.path:
        sys.path.insert(0, _p)

import concourse.bass as bass
import concourse.tile as tile
from concourse import bacc, mybir
from concourse.bass_utils import run_bass_kernel_spmd
from concourse.masks import make_identity

F32 = mybir.dt.float32
I32 = mybir.dt.int32
AF = mybir.ActivationFunctionType
OP = mybir.AluOpType

B, L, H, V = 256, 512, 64, 64
NCORES = 8
BS = B // NCORES          # 32 batches per core
CH = 16                   # t-steps per pipeline chunk
NCH = L // CH             # 32 chunks
LN_EPS = 1e-5


def _ap_bcast(dram_ap, nparts):
    """Partition-broadcast a DRAM AP across nparts partitions."""
    return bass.AP(
        tensor=dram_ap.tensor,
        offset=dram_ap.offset,
        ap=[[0, nparts], *dram_ap.ap],
    )


def build_program():
    nc = bacc.Bacc(None, target_bir_lowering=False)

    seq_p = nc.declare_dram_parameter("seq", [BS, L], I32, isOutput=False)
    embed_p = nc.declare_dram_parameter("embed", [V, H], F32, isOutput=False)
    w1_p = nc.declare_dram_parameter("w1", [H, 2 * H], F32, isOutput=False)
    b1_p = nc.declare_dram_parameter("b1", [2 * H, 1], F32, isOutput=False)
    w2_p = nc.declare_dram_parameter("w2", [2 * H, H], F32, isOutput=False)
    b2_p = nc.declare_dram_parameter("b2", [1, H], F32, isOutput=False)
    ln_g_p = nc.declare_dram_parameter("ln_g", [1, H], F32, isOutput=False)
    ln_b_p = nc.declare_dram_parameter("ln_b", [1, H], F32, isOutput=False)
    rp_w_p = nc.declare_dram_parameter("rp_w", [H, H], F32, isOutput=False)
    rp_b_p = nc.declare_dram_parameter("rp_b", [H, 1], F32, isOutput=False)
    out_w_p = nc.declare_dram_parameter("out_w", [H, V], F32, isOutput=False)
    out_b_p = nc.declare_dram_parameter("out_b", [1, V], F32, isOutput=False)
    out_p = nc.declare_dram_parameter("out", [BS, V], F32, isOutput=True)

    # DRAM bounce for seq^T so per-chunk partition-broadcast DMAs read
    # contiguous runs.
    seqT_d = nc.dram_tensor("seqT_scratch", [L, BS], F32)

    from contextlib import ExitStack

    with tile.TileContext(nc) as tc, ExitStack() as ctx:
        consts = ctx.enter_context(tc.tile_pool(name="consts", bufs=1))
        state = ctx.enter_context(tc.tile_pool(name="state", bufs=1))
        ohp = ctx.enter_context(tc.tile_pool(name="ohp", bufs=4))
        sqp = ctx.enter_context(tc.tile_pool(name="sqp", bufs=4))
        knp = ctx.enter_context(tc.tile_pool(name="knp", bufs=6))
        ps_g = ctx.enter_context(tc.tile_pool(name="ps_g", bufs=2, space="PSUM"))
        ps_ph = ctx.enter_context(tc.tile_pool(name="ps_ph", bufs=1, space="PSUM"))
        ps_m = ctx.enter_context(tc.tile_pool(name="ps_m", bufs=1, space="PSUM"))

        # ---------------- constants / params ----------------
        ident = consts.tile([H, H], F32)
        make_identity(nc, ident)

        eps_sb = consts.tile([V, 1], F32)
        nc.vector.memset(eps_sb, LN_EPS)
        ones1 = consts.tile([1, BS], F32)
        nc.vector.memset(ones1, 1.0)

        viota_i = consts.tile([V, 1], I32)
        nc.gpsimd.iota(viota_i, pattern=[[1, 1]], base=0, channel_multiplier=1)
        viota = consts.tile([V, 1], F32)
        nc.vector.tensor_copy(viota, viota_i)

        # seq load + transpose chain on the SP DMA queue (ahead of params so
        # the per-chunk broadcasts start early); param loads ride the Act
        # engine's DMA queue in parallel.
        seq_hi = consts.tile([BS, 128], I32)
        seq_lo = consts.tile([BS, 384], I32)
        nc.sync.dma_start(out=seq_hi, in_=seq_p[:, 384:512])

        embed_sb = consts.tile([V, H], F32)
        w1_sb = consts.tile([H, 2 * H], F32)
        b1_sb = consts.tile([2 * H, 1], F32)
        w2_sb = consts.tile([2 * H, H], F32)
        rp_w_sb = consts.tile([H, H], F32)
        rp_b_sb = consts.tile([H, 1], F32)
        out_w_sb = consts.tile([H, V], F32)
        out_b_sb = consts.tile([1, V], F32)
        b2_bc = consts.tile([V, H], F32)
        g_bc = consts.tile([V, H], F32)
        bta_bc = consts.tile([V, H], F32)
        # table weights split across the two fast DMA queues so none lands
        # later than ~3us: embed/w1 behind nothing on Act, b1/w2 behind one
        # small seq quarter on SP.
        nc.scalar.dma_start(out=embed_sb, in_=embed_p[:, :])
        nc.scalar.dma_start(out=w1_sb, in_=w1_p[:, :])
        nc.sync.dma_start(out=b1_sb, in_=b1_p[:, :])
        nc.sync.dma_start(out=w2_sb, in_=w2_p[:, :])
        nc.sync.dma_start(out=seq_lo, in_=seq_p[:, 0:384])
        # mid-table broadcast params go through the gpsimd SWDGE queue,
        # which is idle until the first one-hot op.
        nc.gpsimd.dma_start(out=b2_bc, in_=_ap_bcast(b2_p[0, :], V))
        nc.gpsimd.dma_start(out=g_bc, in_=_ap_bcast(ln_g_p[0, :], V))
        nc.gpsimd.dma_start(out=bta_bc, in_=_ap_bcast(ln_b_p[0, :], V))

        # Touch every activation function used later so the ACT table loads
        # (~1.3us each) happen now, overlapped with the DMA transfers --
        # emitted AFTER the dma_start issues so they don't delay them.
        act_warm = consts.tile([V, 1], F32)
        nc.scalar.activation(act_warm, eps_sb, AF.Sqrt)
        nc.scalar.activation(act_warm, eps_sb, AF.Relu)
        nc.scalar.activation(act_warm, eps_sb, AF.Identity, bias=eps_sb[:, 0:1])
        nc.scalar.activation(act_warm, eps_sb, AF.Copy)

        # PE warm-up: a chain of throwaway matmuls keeps the PE p-state ramp
        # going while the parameter DMAs land, so the first gather matmuls run
        # at full clock. Depends only on the gpsimd-built identity.
        dummy_ps = ps_m.tile([BS, BS], F32, tag="psm")
        nc.tensor.matmul(dummy_ps, lhsT=ident[0:BS, 0:BS], rhs=ident[0:BS, 0:BS], start=True, stop=True)
        warm_ps = ps_m.tile([H, H], F32, tag="warm")
        for _ in range(10):
            nc.tensor.matmul(warm_ps, lhsT=ident, rhs=ident, start=True, stop=True)

        # seq -> f32, transpose via PE, bounce to DRAM (all ahead of tables).
        # Processed k descending so the last-time quarter (which the reverse
        # pipeline consumes first) reaches DRAM earliest; the k=3 quarter has
        # its own tiles so it never waits on the k<3 loads.
        seq_fhi = consts.tile([BS, 128], F32)
        seq_flo = consts.tile([BS, 384], F32)
        seqT_hi = consts.tile([128, BS], F32)
        seqT_lo = consts.tile([128, 3, BS], F32)
        nc.vector.tensor_copy(seq_fhi, seq_hi)
        pst = ps_m.tile([128, BS], F32, tag="psm")
        nc.tensor.matmul(pst, lhsT=seq_fhi, rhs=ident[0:BS, 0:BS], start=True, stop=True)
        nc.vector.tensor_copy(seqT_hi, pst)
        nc.sync.dma_start(out=seqT_d[384:512, :], in_=seqT_hi)
        nc.vector.tensor_copy(seq_flo, seq_lo)
        for k in range(2, -1, -1):
            pst = ps_m.tile([128, BS], F32, tag="psm")
            nc.tensor.matmul(pst, lhsT=seq_flo[:, 128 * k:128 * (k + 1)], rhs=ident[0:BS, 0:BS], start=True, stop=True)
            nc.vector.tensor_copy(seqT_lo[:, k, :], pst)
            nc.sync.dma_start(
                out=seqT_d[128 * k:128 * (k + 1), :],
                in_=seqT_lo[:, k, :],
            )

        # ---------------- token tables ----------------
        # eT = embed^T
        psE = ps_m.tile([H, V], F32, tag="psm")
        nc.tensor.matmul(psE, lhsT=embed_sb, rhs=ident, start=True, stop=True)
        eT_sb = consts.tile([H, V], F32)
        nc.scalar.activation(eT_sb, psE, AF.Copy)

        # a1T = (e @ w1)^T  [2H, V], relu(+b1)
        psA = ps_m.tile([2 * H, V], F32, tag="psm")
        nc.tensor.matmul(psA, lhsT=w1_sb, rhs=eT_sb, start=True, stop=True)
        rT = consts.tile([2 * H, V], F32)
        nc.scalar.activation(rT, psA, AF.Relu, bias=b1_sb[:, 0:1])

        # x = e + a1 @ w2 + b2   [V tokens, H]
        psX = ps_m.tile([V, H], F32, tag="psm")
        nc.tensor.matmul(psX, lhsT=rT, rhs=w2_sb, start=True, stop=True)
        x_sb = consts.tile([V, H], F32)
        nc.scalar.activation(x_sb, psX, AF.Copy)
        nc.vector.tensor_add(x_sb, x_sb, embed_sb)
        nc.vector.tensor_add(x_sb, x_sb, b2_bc)

        # LayerNorm over H (free axis)
        st6 = consts.tile([V, 6], F32)
        mv = consts.tile([V, 2], F32)
        nc.vector.bn_stats(st6, x_sb)
        nc.vector.bn_aggr(mv, st6)
        sstd = consts.tile([V, 1], F32)
        rstd = consts.tile([V, 1], F32)
        nc.scalar.activation(sstd, mv[:, 1:2], AF.Sqrt, bias=eps_sb[:, 0:1])
        nc.vector.reciprocal(rstd, sstd)
        T_h = consts.tile([V, H], F32)
        nc.vector.tensor_scalar(
            out=T_h, in0=x_sb, scalar1=mv[:, 0:1], scalar2=rstd[:, 0:1],
            op0=OP.subtract, op1=OP.mult,
        )
        nc.vector.tensor_mul(T_h, T_h, g_bc)
        nc.vector.tensor_add(T_h, T_h, bta_bc)

        # Row norms; T_kn = T_h / max(||T_h||, 1e-12)
        ssq = consts.tile([V, 1], F32)
        scr = consts.tile([V, H], F32)
        nc.vector.scalar_tensor_tensor(
            out=scr, in0=T_h, scalar=1.0, in1=T_h,
            op0=OP.mult, op1=OP.mult, accum_out=ssq[:, 0:1],
        )
        snrm = consts.tile([V, 1], F32)
        nc.scalar.activation(snrm, ssq, AF.Sqrt)
        nc.vector.tensor_scalar(snrm, snrm, 1e-12, None, op0=OP.max)
        rnrm = consts.tile([V, 1], F32)
        nc.vector.reciprocal(rnrm, snrm)

        T_kn = consts.tile([V, H], F32)
        nc.vector.tensor_scalar(
            out=T_kn, in0=T_h, scalar1=rnrm[:, 0:1], scalar2=None,
            op0=OP.mult,
        )

        # stacked identity [128, 32]: row (ph, b) has a 1 in column b.
        # (Filled by DMA later, after the time-critical chunk DMAs are queued.)
        fold_id = consts.tile([128, BS], F32)

        # ---------------- state ----------------
        zneg = state.tile([BS, H], F32)
        u = state.tile([BS, H], F32)
        c_sb = state.tile([BS, L], F32)
        nc.vector.memset(c_sb, 0.0)
        # phased y accumulation: partition (t%4)*32+b, column t//4
        kh_ph = state.tile([128, L // 4, H], F32)   # unnormalized keys, phased
        c_ph = state.tile([128, L // 4], F32)
        y4 = state.tile([128, H], F32)
        u4 = state.tile([128, H], F32)
        y0 = state.tile([BS, H], F32)
        nc.gpsimd.memset(y4, 0.0)

        # ---------------- reverse-order pipeline: gather + scan + y ----------
        for ci in range(NCH - 1, -1, -1):
            t0 = CH * ci

            sqb = sqp.tile([V, CH * BS], F32)
            nc.sync.dma_start(out=sqb, in_=_ap_bcast(seqT_d[t0:t0 + CH, :], V))

            if ci == NCH - 2:
                for sb, p in (
                    (rp_w_sb, rp_w_p), (rp_b_sb, rp_b_p),
                    (out_w_sb, out_w_p), (out_b_sb, out_b_p),
                ):
                    nc.sync.dma_start(out=sb, in_=p[:, :])

            oh = ohp.tile([V, CH, BS], F32)
            nc.gpsimd.tensor_scalar(
                out=oh, in0=sqb.rearrange("v (t b) -> v t b", t=CH), scalar1=viota[:, 0:1], scalar2=None,
                op0=OP.is_equal,
            )

            if ci == NCH - 1:
                # q = h[:, 511, :] (unnormalized), zneg = -q
                psQ = ps_ph.tile([BS, H], F32, tag="psQ")
                nc.tensor.matmul(psQ, lhsT=oh[:, CH - 1, :], rhs=T_h, start=True, stop=True)
                nc.vector.tensor_scalar(
                    out=zneg, in0=psQ, scalar1=-1.0, scalar2=None, op0=OP.mult,
                )

            # gathers in reverse time order; evacuated piecewise (high steps
            # first) so the scan can begin before the rest lands. Quarter
            # granularity for the very first chunk (startup critical path).
            ev = 4 if ci >= NCH - 4 else CH // 2
            psG = ps_g.tile([BS, CH, H], F32, tag="psG")
            knh = knp.tile([BS, CH, H], F32)
            for j in range(CH - 1, -1, -1):
                nc.tensor.matmul(
                    psG[:, j, :], lhsT=oh[:, j, :], rhs=T_kn,
                    start=True, stop=True,
                )
                if j % ev == 0:
                    nc.scalar.activation(
                        knh[:, j:j + ev, :], psG[:, j:j + ev, :], AF.Copy,
                    )

            if ci == 0:
                # unnormalized keys for the final chunk: its y contribution
                # is accumulated on DVE right after the scan (the Pool path's
                # c-rephase DMA would land too late).
                psG0 = ps_g.tile([BS, CH, H], F32, tag="psG")
                kh0 = knp.tile([BS, CH, H], F32)
                for j in range(CH - 1, -1, -1):
                    nc.tensor.matmul(
                        psG0[:, j, :], lhsT=oh[:, j, :], rhs=T_h,
                        start=True, stop=True,
                    )
                nc.scalar.activation(kh0, psG0, AF.Copy)

            # scan steps t = t0+CH-1 .. t0 (t=511 is the query, not a key)
            for j in range(CH - 1, -1, -1):
                t = t0 + j
                if t == L - 1:
                    continue
                kn_ap = knh[:, j, 0:H]
                nc.vector.scalar_tensor_tensor(
                    out=u, in0=kn_ap, scalar=-1.0, in1=zneg,
                    op0=OP.mult, op1=OP.mult, accum_out=c_sb[:, t:t + 1],
                )
                nc.vector.scalar_tensor_tensor(
                    out=zneg, in0=kn_ap, scalar=c_sb[:, t:t + 1], in1=zneg,
                    op0=OP.mult, op1=OP.add,
                )

            # unnormalized keys in phased layout (partition (t%4)*32+b), via
            # one PE matmul per 4 steps; feeds the Pool y-accumulation.
            psP = ps_ph.tile([128, 4, H], F32, tag="psP")
            for g in range(4):
                nc.tensor.matmul(
                    psP[:, g, :],
                    lhsT=oh[:, 4 * g:4 * g + 4, :].rearrange("v t b -> v (t b)"),
                    rhs=T_h, start=True, stop=True,
                )
            nc.scalar.activation(kh_ph[:, 4 * ci:4 * ci + 4, :], psP, AF.Copy)

            if ci == 20:
                # fill fold_id while the SP queue is past its critical phase
                for ph in range(4):
                    nc.sync.dma_start(
                        out=fold_id[32 * ph:32 * (ph + 1), :],
                        in_=ident[0:BS, 0:BS],
                    )

            # every 2 chunks (32 steps; singly for the last two, to shrink
            # the end-of-kernel tail): phase-rearrange c via DMA, then
            # y4 += c_ph[col] * kh_ph[col] on Pool (trailing the scan).
            # c is directly the coefficient of the unnormalized key k_t:
            # y = sum_t c_t k_t (t=511 slot stays zero-filled).
            if ci >= 2 and ci % 2 == 0:
                cols0, ncol, tspan = 4 * ci, 8, 2 * CH
            elif ci == 1:
                cols0, ncol, tspan = 4 * ci, 4, CH
            else:
                cols0 = None
            if cols0 is not None:
                for ph in range(4):
                    nc.sync.dma_start(
                        out=c_ph[32 * ph:32 * (ph + 1), cols0:cols0 + ncol],
                        in_=c_sb[:, t0 + ph:t0 + tspan:4],
                    )
                for col in range(cols0, cols0 + ncol):
                    nc.gpsimd.tensor_scalar(
                        out=u4, in0=kh_ph[:, col, :],
                        scalar1=c_ph[:, col:col + 1], scalar2=None, op0=OP.mult,
                    )
                    nc.gpsimd.tensor_tensor(out=y4, in0=u4, in1=y4, op=OP.add)

            if ci == 0:
                # final chunk's y on DVE (t = 0..15 are all keys)
                nc.vector.tensor_scalar(
                    out=y0, in0=kh0[:, CH - 1, :],
                    scalar1=c_sb[:, CH - 1:CH], scalar2=None, op0=OP.mult,
                )
                for j in range(CH - 2, -1, -1):
                    nc.vector.scalar_tensor_tensor(
                        out=y0, in0=kh0[:, j, :], scalar=c_sb[:, j:j + 1],
                        in1=y0, op0=OP.mult, op1=OP.add,
                    )

        # ---------------- final projections ----------------
        # Precompute the merged projection W_RO = rp_w @ out_w and its bias
        # row B_ROr = rp_b^T out_w + out_b (PE is idle by now), collapsing
        # both output matmuls and the bias adds into one PSUM accumulation.
        psT = ps_m.tile([H, H], F32, tag="psm")
        nc.tensor.matmul(psT, lhsT=rp_w_sb, rhs=ident, start=True, stop=True)
        rp_wT = state.tile([H, H], F32)
        nc.vector.tensor_copy(rp_wT, psT)
        psW = ps_m.tile([H, V], F32, tag="psm")
        nc.tensor.matmul(psW, lhsT=rp_wT, rhs=out_w_sb, start=True, stop=True)
        W_RO = state.tile([H, V], F32)
        nc.vector.tensor_copy(W_RO, psW)
        psB = ps_m.tile([1, V], F32, tag="psm")
        nc.tensor.matmul(psB, lhsT=rp_b_sb, rhs=out_w_sb, start=True, stop=True)
        B_ROr = state.tile([1, V], F32)
        nc.vector.tensor_add(B_ROr, psB, out_b_sb)

        # fold the 4 phase groups of y4 and transpose in one matmul, and
        # accumulate the last chunk's DVE-side y0 (transposed) on top:
        # yT[h, b] = sum_(ph,b') y4[(ph,b'), h] * FOLD[(ph,b'), b] + y0[b, h]
        psF = ps_m.tile([H, BS], F32, tag="psm")
        nc.tensor.matmul(psF, lhsT=y4, rhs=fold_id, start=True, stop=False)
        nc.tensor.matmul(psF, lhsT=y0, rhs=ident[0:BS, 0:BS], start=False, stop=True)
        yT = state.tile([H, BS], F32)
        nc.vector.tensor_copy(yT, psF)

        # out[b, v] = sum_h yT[h, b] W_RO[h, v] + B_ROr[v] -- the bias rides
        # a second contract-1 matmul against a ones column into the same PSUM
        psO = ps_m.tile([BS, V], F32, tag="psm")
        nc.tensor.matmul(psO, lhsT=yT, rhs=W_RO, start=True, stop=False)
        nc.tensor.matmul(psO, lhsT=ones1, rhs=B_ROr, start=False, stop=True)
        o_sb = state.tile([BS, V], F32)
        nc.vector.tensor_copy(o_sb, psO)
        nc.sync.dma_start(out=out_p[:, :], in_=o_sb)

    nc.finalize()
    return nc


_CACHE = {}


def _run(inputs, trace=False, **kw):
    seq = np.asarray(inputs["seq"]).astype(np.int32)
    embed = np.asarray(inputs["embed"], np.float32)
    w1 = np.asarray(inputs["w1"], np.float32)
    b1 = np.asarray(inputs["b1"], np.float32).reshape(2 * H, 1)
    w2 = np.asarray(inputs["w2"], np.float32)
    b2 = np.asarray(inputs["b2"], np.float32).reshape(1, H)
    ln_g = np.asarray(inputs["ln_g"], np.float32).reshape(1, H)
    ln_b = np.asarray(inputs["ln_b"], np.float32).reshape(1, H)
    rp_w = np.asarray(inputs["rp_w"], np.float32)
    rp_b = np.asarray(inputs["rp_b"], np.float32).reshape(H, 1)
    out_w = np.asarray(inputs["out_w"], np.float32)
    out_b = np.asarray(inputs["out_b"], np.float32).reshape(1, V)

    if "prog" not in _CACHE:
        _CACHE["prog"] = build_program()
    nc = _CACHE["prog"]

    in_maps = []
    for c in range(NCORES):
        in_maps.append({
            "seq": seq[BS * c:BS * (c + 1)],
            "embed": embed, "w1": w1, "b1": b1, "w2": w2, "b2": b2,
            "ln_g": ln_g, "ln_b": ln_b,
            "rp_w": rp_w, "rp_b": rp_b, "out_w": out_w, "out_b": out_b,
        })
    br = run_bass_kernel_spmd(nc, in_maps, list(range(NCORES)), trace=trace, **kw)
    out = np.concatenate([r["out"] for r in br.results], axis=0)
    return out, br


def kernel(**inputs) -> np.ndarray:
    return _run(inputs)[0]


# revision 65
# speedup vs baseline: 1.0029x; 1.0010x over previous
"""DeltaNet-style fast-weight kernel for Trainium2 (8 NeuronCores, data-parallel over batch).

Math (per batch element b):
  h_t = LN(e + MLP(e))[seq_t]  -- pure per-token function of seq_t (64 distinct values!)
  keys k_t = h_t, t=0..510 ; kn_t = k_t/||k_t||
  M_t = M_{t-1}(I - kn_t kn_t^T) + k_t kn_t^T ; y = M_510 @ h_511
  out = (y @ rp_w + rp_b) @ out_w + out_b

Key structural reductions vs a naive implementation:
  1. Since h_t depends only on the token id, the entire embed+MLP+LN
     pipeline collapses to 64-row token tables T_h (keys) and T_kn
     (normalized keys) computed once on-chip (64 rows, not B*L tokens).
  2. Per-token key rows are gathered with tiny one-hot PE matmuls
     (lhsT = onehot [64v,32b], rhs = T_kn) into per-16-step chunks,
     produced in REVERSE time order so they pipeline under the scan.
  3. y = sum_t c_t k_t with c_t from the backward vector scan
       z_{510} = q;  c_t = kn_t . z_t;  z_{t-1} = z_t - c_t kn_t
     (z tracked negated; 2 fused DVE ops per step = the critical path;
     ~58+64 DVE cycles per op is the ISA floor for this recurrence).
  4. The y accumulation runs on the otherwise-idle Pool engine in a
     4-phase layout (partition (t%4)*32+b): unnormalized keys T_h[seq_t]
     land there via one extra PE matmul per 4 steps, c is rephased by
     small SBUF-SBUF DMAs, Pool does tensor_scalar+tensor_tensor per
     4 steps, and one PE matmul folds the 4 phases at the end.
"""

import os
import sys

import numpy as np

for _p in ("/opt/trn_rl_repo", "/root/.axon_site/_ro/trn_rl_repo"):
    if os.path.isdir(_p) and _p not in sys.path:
        sys.path.insert(0, _p)

import concourse.bass as bass
import concourse.tile as tile
from concourse import bacc, mybir
from concourse.bass_utils import run_bass_kernel_spmd
from concourse.masks import make_identity

F32 = mybir.dt.float32
I32 = mybir.dt.int32
AF = mybir.ActivationFunctionType
OP = mybir.AluOpType

B, L, H, V = 256, 512, 64, 64
NCORES = 8
BS = B // NCORES          # 32 batches per core
CH = 16                   # t-steps per pipeline chunk
NCH = L // CH             # 32 chunks
LN_EPS = 1e-5


def _ap_bcast(dram_ap, nparts):
    """Partition-broadcast a DRAM AP across nparts partitions."""
    return bass.AP(
        tensor=dram_ap.tensor,
        offset=dram_ap.offset,
        ap=[[0, nparts], *dram_ap.ap],
    )


def build_program():
    nc = bacc.Bacc(None, target_bir_lowering=False)

    seq_p = nc.declare_dram_parameter("seq", [BS, L], I32, isOutput=False)
    embed_p = nc.declare_dram_parameter("embed", [V, H], F32, isOutput=False)
    w1_p = nc.declare_dram_parameter("w1", [H, 2 * H], F32, isOutput=False)
    b1_p = nc.declare_dram_parameter("b1", [2 * H, 1], F32, isOutput=False)
    w2_p = nc.declare_dram_parameter("w2", [2 * H, H], F32, isOutput=False)
    b2_p = nc.declare_dram_parameter("b2", [1, H], F32, isOutput=False)
    ln_g_p = nc.declare_dram_parameter("ln_g", [1, H], F32, isOutput=False)
    ln_b_p = nc.declare_dram_parameter("ln_b", [1, H], F32, isOutput=False)
    rp_w_p = nc.declare_dram_parameter("rp_w", [H, H], F32, isOutput=False)
    rp_b_p = nc.declare_dram_parameter("rp_b", [H, 1], F32, isOutput=False)
    out_w_p = nc.declare_dram_parameter("out_w", [H, V], F32, isOutput=False)
    out_b_p = nc.declare_dram_parameter("out_b", [1, V], F32, isOutput=False)
    out_p = nc.declare_dram_parameter("out", [BS, V], F32, isOutput=True)

    # DRAM bounce for seq^T so per-chunk partition-broadcast DMAs read
    # contiguous runs.
    seqT_d = nc.dram_tensor("seqT_scratch", [L, BS], F32)

    from contextlib import ExitStack

    with tile.TileContext(nc) as tc, ExitStack() as ctx:
        consts = ctx.enter_context(tc.tile_pool(name="consts", bufs=1))
        state = ctx.enter_context(tc.tile_pool(name="state", bufs=1))
        ohp = ctx.enter_context(tc.tile_pool(name="ohp", bufs=4))
        sqp = ctx.enter_context(tc.tile_pool(name="sqp", bufs=4))
        knp = ctx.enter_context(tc.tile_pool(name="knp", bufs=6))
        ps_g = ctx.enter_context(tc.tile_pool(name="ps_g", bufs=2, space="PSUM"))
        ps_ph = ctx.enter_context(tc.tile_pool(name="ps_ph", bufs=1, space="PSUM"))
        ps_m = ctx.enter_context(tc.tile_pool(name="ps_m", bufs=1, space="PSUM"))

        # ---------------- constants / params ----------------
        ident = consts.tile([H, H], F32)
        make_identity(nc, ident)

        eps_sb = consts.tile([V, 1], F32)
        nc.vector.memset(eps_sb, LN_EPS)
        ones1 = consts.tile([1, BS], F32)
        nc.vector.memset(ones1, 1.0)

        viota_i = consts.tile([V, 1], I32)
        nc.gpsimd.iota(viota_i, pattern=[[1, 1]], base=0, channel_multiplier=1)
        viota = consts.tile([V, 1], F32)
        nc.vector.tensor_copy(viota, viota_i)

        # seq load + transpose chain on the SP DMA queue (ahead of params so
        # the per-chunk broadcasts start early); param loads ride the Act
        # engine's DMA queue in parallel.
        seq_hi = consts.tile([BS, 128], I32)
        seq_lo = consts.tile([BS, 384], I32)
        nc.sync.dma_start(out=seq_hi, in_=seq_p[:, 384:512])

        embed_sb = consts.tile([V, H], F32)
        w1_sb = consts.tile([H, 2 * H], F32)
        b1_sb = consts.tile([2 * H, 1], F32)
        w2_sb = consts.tile([2 * H, H], F32)
        rp_w_sb = consts.tile([H, H], F32)
        rp_b_sb = consts.tile([H, 1], F32)
        out_w_sb = consts.tile([H, V], F32)
        out_b_sb = consts.tile([1, V], F32)
        b2_bc = consts.tile([V, H], F32)
        g_bc = consts.tile([V, H], F32)
        bta_bc = consts.tile([V, H], F32)
        # table weights split across the two fast DMA queues so none lands
        # later than ~3us: embed/w1 behind nothing on Act, b1/w2 behind one
        # small seq quarter on SP.
        nc.scalar.dma_start(out=embed_sb, in_=embed_p[:, :])
        nc.scalar.dma_start(out=w1_sb, in_=w1_p[:, :])
        nc.sync.dma_start(out=b1_sb, in_=b1_p[:, :])
        nc.sync.dma_start(out=w2_sb, in_=w2_p[:, :])
        nc.sync.dma_start(out=seq_lo, in_=seq_p[:, 0:384])
        # mid-table broadcast params go through the gpsimd SWDGE queue,
        # which is idle until the first one-hot op.
        nc.gpsimd.dma_start(out=b2_bc, in_=_ap_bcast(b2_p[0, :], V))
        nc.gpsimd.dma_start(out=g_bc, in_=_ap_bcast(ln_g_p[0, :], V))
        nc.gpsimd.dma_start(out=bta_bc, in_=_ap_bcast(ln_b_p[0, :], V))

        # Touch every activation function used later so the ACT table loads
        # (~1.3us each) happen now, overlapped with the DMA transfers --
        # emitted AFTER the dma_start issues so they don't delay them.
        act_warm = consts.tile([V, 1], F32)
        nc.scalar.activation(act_warm, eps_sb, AF.Sqrt)
        nc.scalar.activation(act_warm, eps_sb, AF.Relu)
        nc.scalar.activation(act_warm, eps_sb, AF.Identity, bias=eps_sb[:, 0:1])
        nc.scalar.activation(act_warm, eps_sb, AF.Copy)

        # PE warm-up: a chain of throwaway matmuls keeps the PE p-state ramp
        # going while the parameter DMAs land, so the first gather matmuls run
        # at full clock. Depends only on the gpsimd-built identity.
        dummy_ps = ps_m.tile([BS, BS], F32, tag="psm")
        nc.tensor.matmul(dummy_ps, lhsT=ident[0:BS, 0:BS], rhs=ident[0:BS, 0:BS], start=True, stop=True)
        warm_ps = ps_m.tile([H, H], F32, tag="warm")
        for _ in range(10):
            nc.tensor.matmul(warm_ps, lhsT=ident, rhs=ident, start=True, stop=True)

        # seq -> f32, transpose via PE, bounce to DRAM (all ahead of tables).
        # Processed k descending so the last-time quarter (which the reverse
        # pipeline consumes first) reaches DRAM earliest; the k=3 quarter has
        # its own tiles so it never waits on the k<3 loads.
        seq_fhi = consts.tile([BS, 128], F32)
        seq_flo = consts.tile([BS, 384], F32)
        seqT_hi = consts.tile([128, BS], F32)
        seqT_lo = consts.tile([128, 3, BS], F32)
        nc.vector.tensor_copy(seq_fhi, seq_hi)
        pst = ps_m.tile([128, BS], F32, tag="psm")
        nc.tensor.matmul(pst, lhsT=seq_fhi, rhs=ident[0:BS, 0:BS], start=True, stop=True)
        nc.vector.tensor_copy(seqT_hi, pst)
        # last 16 steps bounce first (the reverse pipeline's first chunk)
        nc.sync.dma_start(out=seqT_d[496:512, :], in_=seqT_hi[112:128, :])
        nc.sync.dma_start(out=seqT_d[384:496, :], in_=seqT_hi[0:112, :])
        nc.vector.tensor_copy(seq_flo, seq_lo)
        for k in range(2, -1, -1):
            pst = ps_m.tile([128, BS], F32, tag="psm")
            nc.tensor.matmul(pst, lhsT=seq_flo[:, 128 * k:128 * (k + 1)], rhs=ident[0:BS, 0:BS], start=True, stop=True)
            nc.vector.tensor_copy(seqT_lo[:, k, :], pst)
            nc.sync.dma_start(
                out=seqT_d[128 * k:128 * (k + 1), :],
                in_=seqT_lo[:, k, :],
            )

        # ---------------- token tables ----------------
        # eT = embed^T
        psE = ps_m.tile([H, V], F32, tag="psm")
        nc.tensor.matmul(psE, lhsT=embed_sb, rhs=ident, start=True, stop=True)
        eT_sb = consts.tile([H, V], F32)
        nc.scalar.activation(eT_sb, psE, AF.Copy)

        # a1T = (e @ w1)^T  [2H, V], relu(+b1)
        psA = ps_m.tile([2 * H, V], F32, tag="psm")
        nc.tensor.matmul(psA, lhsT=w1_sb, rhs=eT_sb, start=True, stop=True)
        rT = consts.tile([2 * H, V], F32)
        nc.scalar.activation(rT, psA, AF.Relu, bias=b1_sb[:, 0:1])

        # x = e + a1 @ w2 + b2   [V tokens, H]
        psX = ps_m.tile([V, H], F32, tag="psm")
        nc.tensor.matmul(psX, lhsT=rT, rhs=w2_sb, start=True, stop=True)
        x_sb = consts.tile([V, H], F32)
        nc.scalar.activation(x_sb, psX, AF.Copy)
        nc.vector.tensor_add(x_sb, x_sb, embed_sb)
        nc.vector.tensor_add(x_sb, x_sb, b2_bc)

        # LayerNorm over H (free axis)
        st6 = consts.tile([V, 6], F32)
        mv = consts.tile([V, 2], F32)
        nc.vector.bn_stats(st6, x_sb)
        nc.vector.bn_aggr(mv, st6)
        sstd = consts.tile([V, 1], F32)
        rstd = consts.tile([V, 1], F32)
        nc.scalar.activation(sstd, mv[:, 1:2], AF.Sqrt, bias=eps_sb[:, 0:1])
        nc.vector.reciprocal(rstd, sstd)
        T_h = consts.tile([V, H], F32)
        nc.vector.tensor_scalar(
            out=T_h, in0=x_sb, scalar1=mv[:, 0:1], scalar2=rstd[:, 0:1],
            op0=OP.subtract, op1=OP.mult,
        )
        nc.vector.tensor_mul(T_h, T_h, g_bc)
        nc.vector.tensor_add(T_h, T_h, bta_bc)

        # Row norms; T_kn = T_h / max(||T_h||, 1e-12)
        ssq = consts.tile([V, 1], F32)
        scr = consts.tile([V, H], F32)
        nc.vector.scalar_tensor_tensor(
            out=scr, in0=T_h, scalar=1.0, in1=T_h,
            op0=OP.mult, op1=OP.mult, accum_out=ssq[:, 0:1],
        )
        snrm = consts.tile([V, 1], F32)
        nc.scalar.activation(snrm, ssq, AF.Sqrt)
        nc.vector.tensor_scalar(snrm, snrm, 1e-12, None, op0=OP.max)
        rnrm = consts.tile([V, 1], F32)
        nc.vector.reciprocal(rnrm, snrm)

        T_kn = consts.tile([V, H], F32)
        nc.vector.tensor_scalar(
            out=T_kn, in0=T_h, scalar1=rnrm[:, 0:1], scalar2=None,
            op0=OP.mult,
        )

        # stacked identity [128, 32]: row (ph, b) has a 1 in column b.
        # (Filled by DMA later, after the time-critical chunk DMAs are queued.)
        fold_id = consts.tile([128, BS], F32)

        # ---------------- state ----------------
        zneg = state.tile([BS, H], F32)
        u = state.tile([BS, H], F32)
        c_sb = state.tile([BS, L], F32)
        nc.vector.memset(c_sb, 0.0)
        # phased y accumulation: partition (t%4)*32+b, column t//4
        kh_ph = state.tile([128, L // 4, H], F32)   # unnormalized keys, phased
        c_ph = state.tile([128, L // 4], F32)
        y4 = state.tile([128, H], F32)
        u4 = state.tile([128, H], F32)
        y0 = state.tile([BS, H], F32)
        nc.gpsimd.memset(y4, 0.0)

        # ---------------- reverse-order pipeline: gather + scan + y ----------
        for ci in range(NCH - 1, -1, -1):
            t0 = CH * ci

            sqb = sqp.tile([V, CH * BS], F32)
            if ci == NCH - 1:
                # half-granularity broadcast for the startup-critical chunk
                for jq in (1, 0):
                    nc.sync.dma_start(
                        out=sqb[:, 8 * BS * jq:8 * BS * (jq + 1)],
                        in_=_ap_bcast(seqT_d[t0 + 8 * jq:t0 + 8 * (jq + 1), :], V),
                    )
            else:
                nc.sync.dma_start(out=sqb, in_=_ap_bcast(seqT_d[t0:t0 + CH, :], V))

            if ci == NCH - 2:
                for sb, p in (
                    (rp_w_sb, rp_w_p), (rp_b_sb, rp_b_p),
                    (out_w_sb, out_w_p), (out_b_sb, out_b_p),
                ):
                    nc.sync.dma_start(out=sb, in_=p[:, :])

            oh = ohp.tile([V, CH, BS], F32)
            if ci == NCH - 1:
                for jq in (1, 0):
                    nc.gpsimd.tensor_scalar(
                        out=oh[:, 8 * jq:8 * (jq + 1), :],
                        in0=sqb[:, 8 * BS * jq:8 * BS * (jq + 1)].rearrange(
                            "v (t b) -> v t b", t=8),
                        scalar1=viota[:, 0:1], scalar2=None,
                        op0=OP.is_equal,
                    )
            else:
                nc.gpsimd.tensor_scalar(
                    out=oh, in0=sqb.rearrange("v (t b) -> v t b", t=CH),
                    scalar1=viota[:, 0:1], scalar2=None,
                    op0=OP.is_equal,
                )

            if ci == NCH - 1:
                # q = h[:, 511, :] (unnormalized), zneg = -q
                psQ = ps_ph.tile([BS, H], F32, tag="psQ")
                nc.tensor.matmul(psQ, lhsT=oh[:, CH - 1, :], rhs=T_h, start=True, stop=True)
                nc.vector.tensor_scalar(
                    out=zneg, in0=psQ, scalar1=-1.0, scalar2=None, op0=OP.mult,
                )

            # gathers in reverse time order; evacuated piecewise (high steps
            # first) so the scan can begin before the rest lands. Quarter
            # granularity for the very first chunk (startup critical path).
            ev = 4 if ci >= NCH - 4 else CH // 2
            psG = ps_g.tile([BS, CH, H], F32, tag="psG")
            knh = knp.tile([BS, CH, H], F32)
            for j in range(CH - 1, -1, -1):
                nc.tensor.matmul(
                    psG[:, j, :], lhsT=oh[:, j, :], rhs=T_kn,
                    start=True, stop=True,
                )
                if j % ev == 0:
                    nc.scalar.activation(
                        knh[:, j:j + ev, :], psG[:, j:j + ev, :], AF.Copy,
                    )

            if ci == 0:
                # unnormalized keys for the final chunk: its y contribution
                # is accumulated on DVE right after the scan (the Pool path's
                # c-rephase DMA would land too late).
                psG0 = ps_g.tile([BS, CH, H], F32, tag="psG")
                kh0 = knp.tile([BS, CH, H], F32)
                for j in range(CH - 1, -1, -1):
                    nc.tensor.matmul(
                        psG0[:, j, :], lhsT=oh[:, j, :], rhs=T_h,
                        start=True, stop=True,
                    )
                nc.scalar.activation(kh0, psG0, AF.Copy)

            # scan steps t = t0+CH-1 .. t0 (t=511 is the query, not a key)
            for j in range(CH - 1, -1, -1):
                t = t0 + j
                if t == L - 1:
                    continue
                kn_ap = knh[:, j, 0:H]
                nc.vector.scalar_tensor_tensor(
                    out=u, in0=kn_ap, scalar=-1.0, in1=zneg,
                    op0=OP.mult, op1=OP.mult, accum_out=c_sb[:, t:t + 1],
                )
                nc.vector.scalar_tensor_tensor(
                    out=zneg, in0=kn_ap, scalar=c_sb[:, t:t + 1], in1=zneg,
                    op0=OP.mult, op1=OP.add,
                )

            # unnormalized keys in phased layout (partition (t%4)*32+b), via
            # one PE matmul per 4 steps; feeds the Pool y-accumulation.
            psP = ps_ph.tile([128, 4, H], F32, tag="psP")
            for g in range(4):
                nc.tensor.matmul(
                    psP[:, g, :],
                    lhsT=oh[:, 4 * g:4 * g + 4, :].rearrange("v t b -> v (t b)"),
                    rhs=T_h, start=True, stop=True,
                )
            nc.scalar.activation(kh_ph[:, 4 * ci:4 * ci + 4, :], psP, AF.Copy)

            if ci == 20:
                # fill fold_id while the SP queue is past its critical phase
                for ph in range(4):
                    nc.sync.dma_start(
                        out=fold_id[32 * ph:32 * (ph + 1), :],
                        in_=ident[0:BS, 0:BS],
                    )

            # every 2 chunks (32 steps; singly for the last two, to shrink
            # the end-of-kernel tail): phase-rearrange c via DMA, then
            # y4 += c_ph[col] * kh_ph[col] on Pool (trailing the scan).
            # c is directly the coefficient of the unnormalized key k_t:
            # y = sum_t c_t k_t (t=511 slot stays zero-filled).
            if ci >= 2 and ci % 2 == 0:
                cols0, ncol, tspan = 4 * ci, 8, 2 * CH
            elif ci == 1:
                cols0, ncol, tspan = 4 * ci, 4, CH
            else:
                cols0 = None
            if cols0 is not None:
                for ph in range(4):
                    nc.sync.dma_start(
                        out=c_ph[32 * ph:32 * (ph + 1), cols0:cols0 + ncol],
                        in_=c_sb[:, t0 + ph:t0 + tspan:4],
                    )
                for col in range(cols0, cols0 + ncol):
                    nc.gpsimd.tensor_scalar(
                        out=u4, in0=kh_ph[:, col, :],
                        scalar1=c_ph[:, col:col + 1], scalar2=None, op0=OP.mult,
                    )
                    nc.gpsimd.tensor_tensor(out=y4, in0=u4, in1=y4, op=OP.add)

            if ci == 0:
                # final chunk's y on DVE (t = 0..15 are all keys)
                nc.vector.tensor_scalar(
                    out=y0, in0=kh0[:, CH - 1, :],
                    scalar1=c_sb[:, CH - 1:CH], scalar2=None, op0=OP.mult,
                )
                for j in range(CH - 2, -1, -1):
                    nc.vector.scalar_tensor_tensor(
                        out=y0, in0=kh0[:, j, :], scalar=c_sb[:, j:j + 1],
                        in1=y0, op0=OP.mult, op1=OP.add,
                    )

        # ---------------- final projections ----------------
        # Precompute the merged projection W_RO = rp_w @ out_w and its bias
        # row B_ROr = rp_b^T out_w + out_b (PE is idle by now), collapsing
        # both output matmuls and the bias adds into one PSUM accumulation.
        psT = ps_m.tile([H, H], F32, tag="psm")
        nc.tensor.matmul(psT, lhsT=rp_w_sb, rhs=ident, start=True, stop=True)
        rp_wT = state.tile([H, H], F32)
        nc.vector.tensor_copy(rp_wT, psT)
        psW = ps_m.tile([H, V], F32, tag="psm")
        nc.tensor.matmul(psW, lhsT=rp_wT, rhs=out_w_sb, start=True, stop=True)
        W_RO = state.tile([H, V], F32)
        nc.vector.tensor_copy(W_RO, psW)
        psB = ps_m.tile([1, V], F32, tag="psm")
        nc.tensor.matmul(psB, lhsT=rp_b_sb, rhs=out_w_sb, start=True, stop=True)
        B_ROr = state.tile([1, V], F32)
        nc.vector.tensor_add(B_ROr, psB, out_b_sb)

        # fold the 4 phase groups of y4 and transpose in one matmul, and
        # accumulate the last chunk's DVE-side y0 (transposed) on top:
        # yT[h, b] = sum_(ph,b') y4[(ph,b'), h] * FOLD[(ph,b'), b] + y0[b, h]
        psF = ps_m.tile([H, BS], F32, tag="psm")
        nc.tensor.matmul(psF, lhsT=y4, rhs=fold_id, start=True, stop=False)
        nc.tensor.matmul(psF, lhsT=y0, rhs=ident[0:BS, 0:BS], start=False, stop=True)
        yT = state.tile([H, BS], F32)
        nc.vector.tensor_copy(yT, psF)

        # out[b, v] = sum_h yT[h, b] W_RO[h, v] + B_ROr[v] -- the bias rides
        # a second contract-1 matmul against a ones column into the same PSUM
        psO = ps_m.tile([BS, V], F32, tag="psm")
        nc.tensor.matmul(psO, lhsT=yT, rhs=W_RO, start=True, stop=False)
        nc.tensor.matmul(psO, lhsT=ones1, rhs=B_ROr, start=False, stop=True)
        o_sb = state.tile([BS, V], F32)
        nc.vector.tensor_copy(o_sb, psO)
        nc.sync.dma_start(out=out_p[:, :], in_=o_sb)

    nc.finalize()
    return nc


_CACHE = {}


def _run(inputs, trace=False, **kw):
    seq = np.asarray(inputs["seq"]).astype(np.int32)
    embed = np.asarray(inputs["embed"], np.float32)
    w1 = np.asarray(inputs["w1"], np.float32)
    b1 = np.asarray(inputs["b1"], np.float32).reshape(2 * H, 1)
    w2 = np.asarray(inputs["w2"], np.float32)
    b2 = np.asarray(inputs["b2"], np.float32).reshape(1, H)
    ln_g = np.asarray(inputs["ln_g"], np.float32).reshape(1, H)
    ln_b = np.asarray(inputs["ln_b"], np.float32).reshape(1, H)
    rp_w = np.asarray(inputs["rp_w"], np.float32)
    rp_b = np.asarray(inputs["rp_b"], np.float32).reshape(H, 1)
    out_w = np.asarray(inputs["out_w"], np.float32)
    out_b = np.asarray(inputs["out_b"], np.float32).reshape(1, V)

    if "prog" not in _CACHE:
        _CACHE["prog"] = build_program()
    nc = _CACHE["prog"]

    in_maps = []
    for c in range(NCORES):
        in_maps.append({
            "seq": seq[BS * c:BS * (c + 1)],
            "embed": embed, "w1": w1, "b1": b1, "w2": w2, "b2": b2,
            "ln_g": ln_g, "ln_b": ln_b,
            "rp_w": rp_w, "rp_b": rp_b, "out_w": out_w, "out_b": out_b,
        })
    br = run_bass_kernel_spmd(nc, in_maps, list(range(NCORES)), trace=trace, **kw)
    out = np.concatenate([r["out"] for r in br.results], axis=0)
    return out, br


def kernel(**inputs) -> np.ndarray:
    return _run(inputs)[0]


# revision 69
# speedup vs baseline: 1.0102x; 1.0073x over previous
"""DeltaNet-style fast-weight kernel for Trainium2 (8 NeuronCores, data-parallel over batch).

Math (per batch element b):
  h_t = LN(e + MLP(e))[seq_t]  -- pure per-token function of seq_t (64 distinct values!)
  keys k_t = h_t, t=0..510 ; kn_t = k_t/||k_t||
  M_t = M_{t-1}(I - kn_t kn_t^T) + k_t kn_t^T ; y = M_510 @ h_511
  out = (y @ rp_w + rp_b) @ out_w + out_b

Key structural reductions vs a naive implementation:
  1. Since h_t depends only on the token id, the entire embed+MLP+LN
     pipeline collapses to 64-row token tables T_h (keys) and T_kn
     (normalized keys) computed once on-chip (64 rows, not B*L tokens).
  2. Per-token key rows are gathered with tiny one-hot PE matmuls
     (lhsT = onehot [64v,32b], rhs = T_kn) into per-16-step chunks,
     produced in REVERSE time order so they pipeline under the scan.
  3. y = sum_t c_t k_t with c_t from the backward vector scan
       z_{510} = q;  c_t = kn_t . z_t;  z_{t-1} = z_t - c_t kn_t
     (z tracked negated; 2 fused DVE ops per step = the critical path;
     ~58+64 DVE cycles per op is the ISA floor for this recurrence).
  4. The y accumulation runs on the otherwise-idle Pool engine in a
     4-phase layout (partition (t%4)*32+b): unnormalized keys T_h[seq_t]
     land there via one extra PE matmul per 4 steps, c is rephased by
     small SBUF-SBUF DMAs, Pool does tensor_scalar+tensor_tensor per
     4 steps, and one PE matmul folds the 4 phases at the end.
"""

import os
import sys

import numpy as np

for _p in ("/opt/trn_rl_repo", "/root/.axon_site/_ro/trn_rl_repo"):
    if os.path.isdir(_p) and _p not in sys.path:
        sys.path.insert(0, _p)

import concourse.bass as bass
import concourse.tile as tile
from concourse import bacc, mybir
from concourse.bass_utils import run_bass_kernel_spmd
from concourse.masks import make_identity

F32 = mybir.dt.float32
I32 = mybir.dt.int32
AF = mybir.ActivationFunctionType
OP = mybir.AluOpType

B, L, H, V = 256, 512, 64, 64
NCORES = 8
BS = B // NCORES          # 32 batches per core
CH = 16                   # t-steps per pipeline chunk
NCH = L // CH             # 32 chunks
LN_EPS = 1e-5


def _ap_bcast(dram_ap, nparts):
    """Partition-broadcast a DRAM AP across nparts partitions."""
    return bass.AP(
        tensor=dram_ap.tensor,
        offset=dram_ap.offset,
        ap=[[0, nparts], *dram_ap.ap],
    )


def build_program():
    nc = bacc.Bacc(None, target_bir_lowering=False)

    seq_p = nc.declare_dram_parameter("seq", [BS, L], I32, isOutput=False)
    embed_p = nc.declare_dram_parameter("embed", [V, H], F32, isOutput=False)
    w1_p = nc.declare_dram_parameter("w1", [H, 2 * H], F32, isOutput=False)
    b1_p = nc.declare_dram_parameter("b1", [2 * H, 1], F32, isOutput=False)
    w2_p = nc.declare_dram_parameter("w2", [2 * H, H], F32, isOutput=False)
    b2_p = nc.declare_dram_parameter("b2", [1, H], F32, isOutput=False)
    ln_g_p = nc.declare_dram_parameter("ln_g", [1, H], F32, isOutput=False)
    ln_b_p = nc.declare_dram_parameter("ln_b", [1, H], F32, isOutput=False)
    rp_w_p = nc.declare_dram_parameter("rp_w", [H, H], F32, isOutput=False)
    rp_b_p = nc.declare_dram_parameter("rp_b", [H, 1], F32, isOutput=False)
    out_w_p = nc.declare_dram_parameter("out_w", [H, V], F32, isOutput=False)
    out_b_p = nc.declare_dram_parameter("out_b", [1, V], F32, isOutput=False)
    out_p = nc.declare_dram_parameter("out", [BS, V], F32, isOutput=True)

    # DRAM bounce for seq^T so per-chunk partition-broadcast DMAs read
    # contiguous runs.
    seqT_d = nc.dram_tensor("seqT_scratch", [L, BS], F32)

    from contextlib import ExitStack

    with tile.TileContext(nc) as tc, ExitStack() as ctx:
        consts = ctx.enter_context(tc.tile_pool(name="consts", bufs=1))
        state = ctx.enter_context(tc.tile_pool(name="state", bufs=1))
        ohp = ctx.enter_context(tc.tile_pool(name="ohp", bufs=4))
        sqp = ctx.enter_context(tc.tile_pool(name="sqp", bufs=4))
        knp = ctx.enter_context(tc.tile_pool(name="knp", bufs=6))
        ps_g = ctx.enter_context(tc.tile_pool(name="ps_g", bufs=2, space="PSUM"))
        ps_ph = ctx.enter_context(tc.tile_pool(name="ps_ph", bufs=1, space="PSUM"))
        ps_m = ctx.enter_context(tc.tile_pool(name="ps_m", bufs=1, space="PSUM"))

        # ---------------- constants / params ----------------
        ident = consts.tile([H, H], F32)
        make_identity(nc, ident)

        eps_sb = consts.tile([V, 1], F32)
        nc.vector.memset(eps_sb, LN_EPS)
        ones1 = consts.tile([1, BS], F32)
        nc.vector.memset(ones1, 1.0)

        viota_i = consts.tile([V, 1], I32)
        nc.gpsimd.iota(viota_i, pattern=[[1, 1]], base=0, channel_multiplier=1)
        viota = consts.tile([V, 1], F32)
        nc.vector.tensor_copy(viota, viota_i)

        # seq load + transpose chain on the SP DMA queue (ahead of params so
        # the per-chunk broadcasts start early); param loads ride the Act
        # engine's DMA queue in parallel.
        seq_hi = consts.tile([BS, 128], I32)
        seq_lo = consts.tile([BS, 384], I32)
        nc.sync.dma_start(out=seq_hi, in_=seq_p[:, 384:512])

        embed_sb = consts.tile([V, H], F32)
        w1_sb = consts.tile([H, 2 * H], F32)
        b1_sb = consts.tile([2 * H, 1], F32)
        w2_sb = consts.tile([2 * H, H], F32)
        rp_w_sb = consts.tile([H, H], F32)
        rp_b_sb = consts.tile([H, 1], F32)
        out_w_sb = consts.tile([H, V], F32)
        out_b_sb = consts.tile([1, V], F32)
        b2_bc = consts.tile([V, H], F32)
        g_bc = consts.tile([V, H], F32)
        bta_bc = consts.tile([V, H], F32)
        # table weights split across the two fast DMA queues so none lands
        # later than ~3us: embed/w1 behind nothing on Act, b1/w2 behind one
        # small seq quarter on SP.
        nc.scalar.dma_start(out=embed_sb, in_=embed_p[:, :])
        nc.scalar.dma_start(out=w1_sb, in_=w1_p[:, :])
        nc.sync.dma_start(out=b1_sb, in_=b1_p[:, :])
        nc.sync.dma_start(out=w2_sb, in_=w2_p[:, :])
        nc.sync.dma_start(out=seq_lo, in_=seq_p[:, 0:384])
        # mid-table broadcast params go through the gpsimd SWDGE queue,
        # which is idle until the first one-hot op.
        nc.gpsimd.dma_start(out=b2_bc, in_=_ap_bcast(b2_p[0, :], V))
        nc.gpsimd.dma_start(out=g_bc, in_=_ap_bcast(ln_g_p[0, :], V))
        nc.gpsimd.dma_start(out=bta_bc, in_=_ap_bcast(ln_b_p[0, :], V))

        # Touch every activation function used later so the ACT table loads
        # (~1.3us each) happen now, overlapped with the DMA transfers --
        # emitted AFTER the dma_start issues so they don't delay them.
        act_warm = consts.tile([V, 1], F32)
        nc.scalar.activation(act_warm, eps_sb, AF.Sqrt)
        nc.scalar.activation(act_warm, eps_sb, AF.Relu)
        nc.scalar.activation(act_warm, eps_sb, AF.Identity, bias=eps_sb[:, 0:1])
        nc.scalar.activation(act_warm, eps_sb, AF.Copy)

        # PE warm-up: a chain of throwaway matmuls keeps the PE p-state ramp
        # going while the parameter DMAs land, so the first gather matmuls run
        # at full clock. Depends only on the gpsimd-built identity.
        dummy_ps = ps_m.tile([BS, BS], F32, tag="psm")
        nc.tensor.matmul(dummy_ps, lhsT=ident[0:BS, 0:BS], rhs=ident[0:BS, 0:BS], start=True, stop=True)
        warm_ps = ps_m.tile([H, H], F32, tag="warm")
        for _ in range(10):
            nc.tensor.matmul(warm_ps, lhsT=ident, rhs=ident, start=True, stop=True)

        # seq -> f32, transpose via PE, bounce to DRAM (all ahead of tables).
        # Processed k descending so the last-time quarter (which the reverse
        # pipeline consumes first) reaches DRAM earliest; the k=3 quarter has
        # its own tiles so it never waits on the k<3 loads.
        seq_fhi = consts.tile([BS, 128], F32)
        seq_flo = consts.tile([BS, 384], F32)
        seqT_hi = consts.tile([128, BS], F32)
        seqT_lo = consts.tile([128, 3, BS], F32)
        nc.vector.tensor_copy(seq_fhi, seq_hi)
        pst = ps_m.tile([128, BS], F32, tag="psm")
        nc.tensor.matmul(pst, lhsT=seq_fhi, rhs=ident[0:BS, 0:BS], start=True, stop=True)
        nc.vector.tensor_copy(seqT_hi, pst)
        # last 16 steps bounce first (the reverse pipeline's first chunk)
        nc.sync.dma_start(out=seqT_d[496:512, :], in_=seqT_hi[112:128, :])
        nc.sync.dma_start(out=seqT_d[384:496, :], in_=seqT_hi[0:112, :])
        nc.vector.tensor_copy(seq_flo, seq_lo)
        for k in range(2, -1, -1):
            pst = ps_m.tile([128, BS], F32, tag="psm")
            nc.tensor.matmul(pst, lhsT=seq_flo[:, 128 * k:128 * (k + 1)], rhs=ident[0:BS, 0:BS], start=True, stop=True)
            nc.vector.tensor_copy(seqT_lo[:, k, :], pst)
            nc.sync.dma_start(
                out=seqT_d[128 * k:128 * (k + 1), :],
                in_=seqT_lo[:, k, :],
            )

        # ---------------- token tables ----------------
        # eT = embed^T
        psE = ps_m.tile([H, V], F32, tag="psm")
        nc.tensor.matmul(psE, lhsT=embed_sb, rhs=ident, start=True, stop=True)
        eT_sb = consts.tile([H, V], F32)
        nc.scalar.activation(eT_sb, psE, AF.Copy)

        # a1T = (e @ w1)^T  [2H, V], relu(+b1)
        psA = ps_m.tile([2 * H, V], F32, tag="psm")
        nc.tensor.matmul(psA, lhsT=w1_sb, rhs=eT_sb, start=True, stop=True)
        rT = consts.tile([2 * H, V], F32)
        nc.scalar.activation(rT, psA, AF.Relu, bias=b1_sb[:, 0:1])

        # x = e + a1 @ w2 + b2   [V tokens, H]
        psX = ps_m.tile([V, H], F32, tag="psm")
        nc.tensor.matmul(psX, lhsT=rT, rhs=w2_sb, start=True, stop=True)
        x_sb = consts.tile([V, H], F32)
        nc.scalar.activation(x_sb, psX, AF.Copy)
        nc.vector.tensor_add(x_sb, x_sb, embed_sb)
        nc.vector.tensor_add(x_sb, x_sb, b2_bc)

        # LayerNorm over H (free axis)
        st6 = consts.tile([V, 6], F32)
        mv = consts.tile([V, 2], F32)
        nc.vector.bn_stats(st6, x_sb)
        nc.vector.bn_aggr(mv, st6)
        sstd = consts.tile([V, 1], F32)
        rstd = consts.tile([V, 1], F32)
        nc.scalar.activation(sstd, mv[:, 1:2], AF.Sqrt, bias=eps_sb[:, 0:1])
        nc.vector.reciprocal(rstd, sstd)
        T_h = consts.tile([V, H], F32)
        nc.vector.tensor_scalar(
            out=T_h, in0=x_sb, scalar1=mv[:, 0:1], scalar2=rstd[:, 0:1],
            op0=OP.subtract, op1=OP.mult,
        )
        nc.vector.tensor_mul(T_h, T_h, g_bc)
        nc.vector.tensor_add(T_h, T_h, bta_bc)

        # Row norms; T_kn = T_h / max(||T_h||, 1e-12)
        ssq = consts.tile([V, 1], F32)
        scr = consts.tile([V, H], F32)
        nc.vector.scalar_tensor_tensor(
            out=scr, in0=T_h, scalar=1.0, in1=T_h,
            op0=OP.mult, op1=OP.mult, accum_out=ssq[:, 0:1],
        )
        snrm = consts.tile([V, 1], F32)
        nc.scalar.activation(snrm, ssq, AF.Sqrt)
        nc.vector.tensor_scalar(snrm, snrm, 1e-12, None, op0=OP.max)
        rnrm = consts.tile([V, 1], F32)
        nc.vector.reciprocal(rnrm, snrm)

        T_kn = consts.tile([V, H], F32)
        nc.vector.tensor_scalar(
            out=T_kn, in0=T_h, scalar1=rnrm[:, 0:1], scalar2=None,
            op0=OP.mult,
        )

        # stacked identity [128, 32]: row (ph, b) has a 1 in column b.
        # (Filled by DMA later, after the time-critical chunk DMAs are queued.)
        fold_id = consts.tile([128, BS], F32)

        # ---------------- state ----------------
        zneg = state.tile([BS, H], F32)
        u = state.tile([BS, H], F32)
        c_sb = state.tile([BS, L], F32)
        nc.vector.memset(c_sb, 0.0)
        # phased y accumulation: partition (t%4)*32+b, column t//4
        kh_ph = state.tile([128, L // 4, H], F32)   # unnormalized keys, phased
        c_ph = state.tile([128, L // 4], F32)
        y4 = state.tile([128, H], F32)
        u4 = state.tile([128, H], F32)
        y0 = state.tile([BS, H], F32)
        nc.gpsimd.memset(y4, 0.0)

        # ---------------- reverse-order pipeline: gather + scan + y ----------
        for ci in range(NCH - 1, -1, -1):
            t0 = CH * ci

            sqb = sqp.tile([V, CH * BS], F32)
            if ci == NCH - 1:
                # half-granularity broadcast for the startup-critical chunk
                for jq in (1, 0):
                    nc.sync.dma_start(
                        out=sqb[:, 8 * BS * jq:8 * BS * (jq + 1)],
                        in_=_ap_bcast(seqT_d[t0 + 8 * jq:t0 + 8 * (jq + 1), :], V),
                    )
            else:
                nc.sync.dma_start(out=sqb, in_=_ap_bcast(seqT_d[t0:t0 + CH, :], V))

            if ci == NCH - 2:
                for sb, p in (
                    (rp_w_sb, rp_w_p), (rp_b_sb, rp_b_p),
                    (out_w_sb, out_w_p), (out_b_sb, out_b_p),
                ):
                    nc.sync.dma_start(out=sb, in_=p[:, :])

            oh = ohp.tile([V, CH, BS], F32)
            if ci == NCH - 1:
                for jq in (1, 0):
                    nc.gpsimd.tensor_scalar(
                        out=oh[:, 8 * jq:8 * (jq + 1), :],
                        in0=sqb[:, 8 * BS * jq:8 * BS * (jq + 1)].rearrange(
                            "v (t b) -> v t b", t=8),
                        scalar1=viota[:, 0:1], scalar2=None,
                        op0=OP.is_equal,
                    )
            else:
                nc.gpsimd.tensor_scalar(
                    out=oh, in0=sqb.rearrange("v (t b) -> v t b", t=CH),
                    scalar1=viota[:, 0:1], scalar2=None,
                    op0=OP.is_equal,
                )

            if ci == NCH - 1:
                # q = h[:, 511, :] (unnormalized), zneg = -q
                psQ = ps_ph.tile([BS, H], F32, tag="psQ")
                nc.tensor.matmul(psQ, lhsT=oh[:, CH - 1, :], rhs=T_h, start=True, stop=True)
                nc.vector.tensor_scalar(
                    out=zneg, in0=psQ, scalar1=-1.0, scalar2=None, op0=OP.mult,
                )

            # gathers in reverse time order; evacuated piecewise (high steps
            # first) so the scan can begin before the rest lands. Quarter
            # granularity for the very first chunk (startup critical path).
            ev = 4 if ci >= NCH - 4 else CH // 2
            psG = ps_g.tile([BS, CH, H], F32, tag="psG")
            knh = knp.tile([BS, CH, H], F32)
            for j in range(CH - 1, -1, -1):
                nc.tensor.matmul(
                    psG[:, j, :], lhsT=oh[:, j, :], rhs=T_kn,
                    start=True, stop=True,
                )
                if j % ev == 0:
                    nc.scalar.activation(
                        knh[:, j:j + ev, :], psG[:, j:j + ev, :], AF.Copy,
                    )

            if ci == 0:
                # unnormalized keys for the final 8 steps: their y is
                # accumulated on DVE right after the scan (the Pool path's
                # c-rephase DMA would land too late for t < 8).
                psG0 = ps_g.tile([BS, CH // 2, H], F32, tag="psG")
                kh0 = knp.tile([BS, CH // 2, H], F32)
                for j in range(CH // 2 - 1, -1, -1):
                    nc.tensor.matmul(
                        psG0[:, j, :], lhsT=oh[:, j, :], rhs=T_h,
                        start=True, stop=True,
                    )
                nc.scalar.activation(kh0, psG0, AF.Copy)

            # unnormalized keys in phased layout (partition (t%4)*32+b), via
            # one PE matmul per 4 steps; feeds the Pool y-accumulation.
            psP = ps_ph.tile([128, 4, H], F32, tag="psP")
            for g in range(4):
                nc.tensor.matmul(
                    psP[:, g, :],
                    lhsT=oh[:, 4 * g:4 * g + 4, :].rearrange("v t b -> v (t b)"),
                    rhs=T_h, start=True, stop=True,
                )
            nc.scalar.activation(kh_ph[:, 4 * ci:4 * ci + 4, :], psP, AF.Copy)

            # scan steps t = t0+CH-1 .. t0 (t=511 is the query, not a key)
            for j in range(CH - 1, -1, -1):
                t = t0 + j
                if t == L - 1:
                    continue
                kn_ap = knh[:, j, 0:H]
                nc.vector.scalar_tensor_tensor(
                    out=u, in0=kn_ap, scalar=-1.0, in1=zneg,
                    op0=OP.mult, op1=OP.mult, accum_out=c_sb[:, t:t + 1],
                )
                nc.vector.scalar_tensor_tensor(
                    out=zneg, in0=kn_ap, scalar=c_sb[:, t:t + 1], in1=zneg,
                    op0=OP.mult, op1=OP.add,
                )
                if ci == 0 and j == CH // 2:
                    # c for t=8..15 is final: route those four-step columns
                    # through the Pool path now -- the rephase DMA lands while
                    # the last 8 scan steps still run.
                    for ph in range(4):
                        nc.sync.dma_start(
                            out=c_ph[32 * ph:32 * (ph + 1), 2:4],
                            in_=c_sb[:, 8 + ph:16:4],
                        )
                    for col in (2, 3):
                        nc.gpsimd.tensor_scalar(
                            out=u4, in0=kh_ph[:, col, :],
                            scalar1=c_ph[:, col:col + 1], scalar2=None,
                            op0=OP.mult,
                        )
                        nc.gpsimd.tensor_tensor(out=y4, in0=u4, in1=y4, op=OP.add)

            if ci == 20:
                # fill fold_id while the SP queue is past its critical phase
                for ph in range(4):
                    nc.sync.dma_start(
                        out=fold_id[32 * ph:32 * (ph + 1), :],
                        in_=ident[0:BS, 0:BS],
                    )

            # every 2 chunks (32 steps; singly for the last two, to shrink
            # the end-of-kernel tail): phase-rearrange c via DMA, then
            # y4 += c_ph[col] * kh_ph[col] on Pool (trailing the scan).
            # c is directly the coefficient of the unnormalized key k_t:
            # y = sum_t c_t k_t (t=511 slot stays zero-filled).
            if ci >= 2 and ci % 2 == 0:
                cols0, ncol, tspan = 4 * ci, 8, 2 * CH
            elif ci == 1:
                cols0, ncol, tspan = 4 * ci, 4, CH
            else:
                cols0 = None
            if cols0 is not None:
                for ph in range(4):
                    nc.sync.dma_start(
                        out=c_ph[32 * ph:32 * (ph + 1), cols0:cols0 + ncol],
                        in_=c_sb[:, t0 + ph:t0 + tspan:4],
                    )
                for col in range(cols0, cols0 + ncol):
                    nc.gpsimd.tensor_scalar(
                        out=u4, in0=kh_ph[:, col, :],
                        scalar1=c_ph[:, col:col + 1], scalar2=None, op0=OP.mult,
                    )
                    nc.gpsimd.tensor_tensor(out=y4, in0=u4, in1=y4, op=OP.add)

            if ci == 0:
                # final 8 steps' y on DVE (their c lands after the scan ends)
                nc.vector.tensor_scalar(
                    out=y0, in0=kh0[:, CH // 2 - 1, :],
                    scalar1=c_sb[:, CH // 2 - 1:CH // 2], scalar2=None,
                    op0=OP.mult,
                )
                for j in range(CH // 2 - 2, -1, -1):
                    nc.vector.scalar_tensor_tensor(
                        out=y0, in0=kh0[:, j, :], scalar=c_sb[:, j:j + 1],
                        in1=y0, op0=OP.mult, op1=OP.add,
                    )

        # ---------------- final projections ----------------
        # Precompute the merged projection W_RO = rp_w @ out_w and its bias
        # row B_ROr = rp_b^T out_w + out_b (PE is idle by now), collapsing
        # both output matmuls and the bias adds into one PSUM accumulation.
        psT = ps_m.tile([H, H], F32, tag="psm")
        nc.tensor.matmul(psT, lhsT=rp_w_sb, rhs=ident, start=True, stop=True)
        rp_wT = state.tile([H, H], F32)
        nc.vector.tensor_copy(rp_wT, psT)
        psW = ps_m.tile([H, V], F32, tag="psm")
        nc.tensor.matmul(psW, lhsT=rp_wT, rhs=out_w_sb, start=True, stop=True)
        W_RO = state.tile([H, V], F32)
        nc.vector.tensor_copy(W_RO, psW)
        psB = ps_m.tile([1, V], F32, tag="psm")
        nc.tensor.matmul(psB, lhsT=rp_b_sb, rhs=out_w_sb, start=True, stop=True)
        B_ROr = state.tile([1, V], F32)
        nc.vector.tensor_add(B_ROr, psB, out_b_sb)

        # fold the 4 phase groups of y4 and transpose in one matmul, and
        # accumulate the last chunk's DVE-side y0 (transposed) on top:
        # yT[h, b] = sum_(ph,b') y4[(ph,b'), h] * FOLD[(ph,b'), b] + y0[b, h]
        psF = ps_m.tile([H, BS], F32, tag="psm")
        nc.tensor.matmul(psF, lhsT=y4, rhs=fold_id, start=True, stop=False)
        nc.tensor.matmul(psF, lhsT=y0, rhs=ident[0:BS, 0:BS], start=False, stop=True)
        yT = state.tile([H, BS], F32)
        nc.vector.tensor_copy(yT, psF)

        # out[b, v] = sum_h yT[h, b] W_RO[h, v] + B_ROr[v] -- the bias rides
        # a second contract-1 matmul against a ones column into the same PSUM
        psO = ps_m.tile([BS, V], F32, tag="psm")
        nc.tensor.matmul(psO, lhsT=yT, rhs=W_RO, start=True, stop=False)
        nc.tensor.matmul(psO, lhsT=ones1, rhs=B_ROr, start=False, stop=True)
        o_sb = state.tile([BS, V], F32)
        nc.vector.tensor_copy(o_sb, psO)
        nc.sync.dma_start(out=out_p[:, :], in_=o_sb)

    nc.finalize()
    return nc


_CACHE = {}


def _run(inputs, trace=False, **kw):
    seq = np.asarray(inputs["seq"]).astype(np.int32)
    embed = np.asarray(inputs["embed"], np.float32)
    w1 = np.asarray(inputs["w1"], np.float32)
    b1 = np.asarray(inputs["b1"], np.float32).reshape(2 * H, 1)
    w2 = np.asarray(inputs["w2"], np.float32)
    b2 = np.asarray(inputs["b2"], np.float32).reshape(1, H)
    ln_g = np.asarray(inputs["ln_g"], np.float32).reshape(1, H)
    ln_b = np.asarray(inputs["ln_b"], np.float32).reshape(1, H)
    rp_w = np.asarray(inputs["rp_w"], np.float32)
    rp_b = np.asarray(inputs["rp_b"], np.float32).reshape(H, 1)
    out_w = np.asarray(inputs["out_w"], np.float32)
    out_b = np.asarray(inputs["out_b"], np.float32).reshape(1, V)

    if "prog" not in _CACHE:
        _CACHE["prog"] = build_program()
    nc = _CACHE["prog"]

    in_maps = []
    for c in range(NCORES):
        in_maps.append({
            "seq": seq[BS * c:BS * (c + 1)],
            "embed": embed, "w1": w1, "b1": b1, "w2": w2, "b2": b2,
            "ln_g": ln_g, "ln_b": ln_b,
            "rp_w": rp_w, "rp_b": rp_b, "out_w": out_w, "out_b": out_b,
        })
    br = run_bass_kernel_spmd(nc, in_maps, list(range(NCORES)), trace=trace, **kw)
    out = np.concatenate([r["out"] for r in br.results], axis=0)
    return out, br


def kernel(**inputs) -> np.ndarray:
    return _run(inputs)[0]
